# revision 50
# baseline (speedup 1.0000x reference)
"""BNN Linear + BatchNorm (training-mode stats) Trainium2 kernel.

out = BN(sign(x) @ sign(W).T), batch stats over the full 8192-row batch,
data-parallel over 8 NeuronCores (1024 batch rows per core).

Per-core pipeline (SPMD, one program on all cores):
  1. f32->bf16 casts run as DRAM->DRAM SWDGE DMAs; all operand transposes are
     xbar DMA-transposes *directly from DRAM* (a handful of large ops, all
     emitted before any collective, since Tile serializes DMA-transposes
     against both DMA copies and collectives).  sign() is applied after the
     transpose -- it is elementwise, so layout-agnostic.
  2. Weight prep is sharded: each core casts+transposes only its 256 rows of
     W, signs them, and the bf16 [IN_p, k, out] shard tiles are AllGathered
     (1 MB -> 8 MB).  m-tiles 0..1 are additionally prepped locally from a
     replicated "w_head" input so the first matmuls don't wait on the
     AllGather.
  3. GEMM: per m (16 OUT tiles) x h (2 batch chunks of 512): accumulate 16
     matmuls (k) into PSUM.  bf16 is exact for {-1,0,+1}; fp32 PSUM
     accumulation keeps results integer-exact.
  4. Drain PSUM -> raw f32 ([OUT_p, batch_f] layout) via ScalarE copy with
     fused row-sum (accum_out); sum-of-squares via DVE tensor_tensor_reduce.
  5. BN stats AllReduce is split (m 0..11 / m 12..15) so phase-A
     normalize/store overlaps phase-B GEMM; only the small second AllReduce
     is exposed at the tail.
  6. Normalize (ScalarE Identity with per-partition scale/bias), DVE 32x32
     stream-transpose, block-permuting DMA store to the [batch, OUT] layout.
"""

import os
import numpy as np
from contextlib import ExitStack

import concourse.bass as bass
import concourse.mybir as mybir
import concourse.tile as tile
from concourse import bacc
from concourse import bass_utils

F32 = mybir.dt.float32
BF16 = mybir.dt.bfloat16
AF = mybir.ActivationFunctionType
ALU = mybir.AluOpType

N_CORES = 8
B_FULL = 8192
IN = 2048
OUT = 2048
P = 128
BS = B_FULL // N_CORES       # 1024 batch rows per core
NK = IN // P                 # 16 contraction tiles
NM = OUT // P                # 16 output-channel tiles
MPC = NM // N_CORES          # 2 m-tiles prepped per core for the AllGather
WPC = OUT // N_CORES         # 256 weight rows per core
CHUNK = 512                  # PSUM free width (one f32 bank)
NH = BS // CHUNK             # 2 batch chunks
PHASES = [list(range(0, 8)), list(range(8, 14)), list(range(14, 16))]
NLM = 16                     # m-tiles prepped locally from w_head
WHR = NLM * P                # w_head rows
BN_EPS = 1e-5


def _body(nc, tc, x_ap, w_ap, whead_ap, gamma_ap, beta_ap, out_ap,
          do_gemm=True, do_drain=True, do_ar=True, do_tail=True,
          nlm=NLM, psum_bufs=8):
    ctx = ExitStack()
    with ctx:
        wt_pool = ctx.enter_context(tc.tile_pool(name="wt_pool", bufs=4))
        psum_pool = ctx.enter_context(
            tc.tile_pool(name="psum", bufs=psum_bufs, space="PSUM"))
        dmy_pool = ctx.enter_context(tc.tile_pool(name="dmy", bufs=1))
        norm_pool = ctx.enter_context(tc.tile_pool(name="norm", bufs=3))
        tp_pool = ctx.enter_context(tc.tile_pool(name="tp", bufs=3))
        persist = ctx.enter_context(tc.tile_pool(name="persist", bufs=1))
        dram = ctx.enter_context(tc.tile_pool(name="dram", bufs=1, space="DRAM"))

        # ---------- DRAM bf16 staging (casting DMAs) ----------
        # Ordered for fastest availability of (xTa, wt_g0): the single DMA
        # device serializes everything, so front-load what the first matmuls
        # need.  xbar transpose maps in[c, t*128+p] -> out[p, t, c].
        xbf = dram.tile([BS, IN], BF16, name="xbf")
        wbf_sh = dram.tile([WPC, IN], BF16, name="wbf_sh")
        wbf_hd = dram.tile([nlm * P, IN], BF16, name="wbf_hd")
        wt_shard = persist.tile([P, MPC, NK, P], BF16, name="wt_shard")
        wt_g0 = persist.tile([P, nlm, NK, P], BF16, name="wt_g0")
        xTh = [
            persist.tile([P, NK, CHUNK], BF16, name="xTa"),
            persist.tile([P, NK, CHUNK], BF16, name="xTb"),
        ]

        def x_quarter(q):
            nc.gpsimd.dma_start(
                xbf[q * 256:(q + 1) * 256, :], x_ap[q * 256:(q + 1) * 256, :])
            sl = xTh[q // 2][:, :, (q % 2) * 256:(q % 2 + 1) * 256]
            nc.sync.dma_start_transpose(sl, xbf[q * 256:(q + 1) * 256, :])
            nc.scalar.sign(sl, sl)

        # first GEMM inputs: interleave w_head (per-mi casts) with x half a
        def whead_mi(mi):
            nc.gpsimd.dma_start(
                wbf_hd[mi * P:(mi + 1) * P, :], whead_ap[mi * P:(mi + 1) * P, :])
            nc.sync.dma_start_transpose(
                wt_g0[:, mi, :, :], wbf_hd[mi * P:(mi + 1) * P, :])
            sl = wt_g0[:, mi, :, :]
            nc.scalar.sign(sl, sl)

        whead_mi(0)
        x_quarter(0)
        whead_mi(1)
        x_quarter(1)
        for mi in range(2, min(nlm, 3)):
            whead_mi(mi)
        x_quarter(2)
        for mi in range(3, nlm):
            whead_mi(mi)
        x_quarter(3)

        # ---------- AllGather of the (unsigned, untransposed) bf16 shard ----
        # The f32->bf16 cast DMA writes straight into the AllGather input;
        # per-m DRAM-source transposes + sign run during the GEMM.
        ag_out = None
        if nlm < NM:
            ag_in = dram.tile([WPC, IN], BF16, name="ag_in")
            ag_out = dram.tile([N_CORES, WPC, IN], BF16, name="ag_out",
                               addr_space="Shared")
            nc.gpsimd.dma_start(ag_in[:], w_ap)        # cast f32 -> bf16
            nc.gpsimd.collective_compute(
                "AllGather", ALU.bypass,
                replica_groups=[list(range(N_CORES))],
                ins=[ag_in[:].opt()],
                outs=[ag_out[:].opt()],
            )

        # ---------- constants ----------
        gamma_t = persist.tile([P, NM], F32, name="gamma_t")
        beta_t = persist.tile([P, NM], F32, name="beta_t")
        nc.gpsimd.dma_start(gamma_t[:], gamma_ap.rearrange("(m p) -> p m", p=P))
        nc.gpsimd.dma_start(beta_t[:], beta_ap.rearrange("(m p) -> p m", p=P))
        eps_t = persist.tile([P, 1], F32, name="eps_t")
        nc.vector.memset(eps_t[:], BN_EPS)

        # ---------- per-phase state ----------
        phase_m = PHASES
        phase_of = {}
        for _ph, _ms in enumerate(phase_m):
            for _m in _ms:
                phase_of[_m] = _ph
        rawp = [
            persist.tile([P, len(ms), BS], F32, name=f"raw{ph}")
            for ph, ms in enumerate(phase_m)
        ]
        sums_p = [
            persist.tile([P, len(ms) * NH], F32, name=f"sums_p{ph}")
            for ph, ms in enumerate(phase_m)
        ]
        sumsq_p = [
            persist.tile([P, len(ms) * NH], F32, name=f"sumsq_p{ph}")
            for ph, ms in enumerate(phase_m)
        ]

        # ---------- GEMM ----------
        wt_cache = {}

        def mm_chunk(m, h):
            ph = phase_of[m]
            mi = m - phase_m[ph][0]
            if m not in wt_cache:
                if m < nlm:
                    wt_cache[m] = lambda k, mw=m: wt_g0[:, mw, k, :]
                else:
                    wTm = wt_pool.tile([P, NK, P], BF16, name="wTm")
                    nc.sync.dma_start_transpose(
                        wTm[:],
                        ag_out[m // MPC, (m % MPC) * P:(m % MPC + 1) * P, :])
                    nc.scalar.sign(wTm[:], wTm[:])
                    wt_cache[m] = lambda k, t=wTm: t[:, k, :]
            lh = wt_cache[m]
            ps = psum_pool.tile([P, CHUNK], F32, name="ps")
            for k in range(NK):
                nc.tensor.matmul(
                    ps[:],
                    lhsT=lh(k),
                    rhs=xTh[h][:, k, :],
                    start=(k == 0),
                    stop=(k == NK - 1),
                )
            if not do_drain:
                return
            col = mi * NH + h
            raw_sl = rawp[ph][:, mi, h * CHUNK:(h + 1) * CHUNK]
            nc.scalar.copy(raw_sl, ps[:])
            nc.vector.tensor_reduce(
                sums_p[ph][:, col:col + 1], raw_sl,
                axis=mybir.AxisListType.X, op=ALU.add,
            )
            dmy = dmy_pool.tile([P, CHUNK], F32, name="dmy")
            nc.vector.tensor_mul(dmy[:], raw_sl, raw_sl)
            nc.vector.tensor_reduce(
                sumsq_p[ph][:, col:col + 1], dmy[:],
                axis=mybir.AxisListType.X, op=ALU.add,
            )

        def gemm_all(emit_tail):
            # h0 chunks of m0/m1 first: xTb and the AllGather results arrive
            # a little after xTa/wt_g0, so don't demand them immediately.
            order = [(0, 0), (1, 0), (0, 1), (1, 1)]
            order += [(m, h) for m in range(2, NM) for h in range(NH)]
            done = set()
            for m, h in order:
                mm_chunk(m, h)
                done.add((m, h))
                # emit each phase's stats+tail as soon as its chunks are in:
                # engine queues execute in (scheduled ~ emission) order, so
                # this is what lets tail work overlap later-phase GEMM.
                for ph, ms in enumerate(phase_m):
                    if emit_tail and ph not in emitted and all(
                            (mm, hh) in done for mm in ms for hh in range(NH)):
                        emitted.add(ph)
                        stats_and_tail(ph)

        # ---------- stats AllReduce + normalize + store, per phase ----------
        def stats_and_tail(ph):
            nm_ph = len(phase_m[ph])
            stats_loc = persist.tile([P, 2 * nm_ph], F32, name=f"stats_loc{ph}")
            stats_glob = persist.tile([P, 2 * nm_ph], F32, name=f"stats_glob{ph}")
            cc_in = dram.tile([P, 2 * nm_ph], F32, name=f"cc_in{ph}")
            cc_out = dram.tile([P, 2 * nm_ph], F32, name=f"cc_out{ph}",
                               addr_space="Shared")

            nc.vector.tensor_reduce(
                stats_loc[:, 0:nm_ph],
                sums_p[ph][:].rearrange("p (m h) -> p m h", h=NH),
                axis=mybir.AxisListType.X, op=ALU.add)
            nc.vector.tensor_reduce(
                stats_loc[:, nm_ph:],
                sumsq_p[ph][:].rearrange("p (m h) -> p m h", h=NH),
                axis=mybir.AxisListType.X, op=ALU.add)
            nc.gpsimd.dma_start(cc_in[:], stats_loc[:])
            nc.gpsimd.collective_compute(
                "AllReduce", ALU.add,
                replica_groups=[list(range(N_CORES))],
                ins=[cc_in[:].opt()],
                outs=[cc_out[:].opt()],
            )
            nc.gpsimd.dma_start(stats_glob[:], cc_out[:])

            mean_t = persist.tile([P, nm_ph], F32, name=f"mean{ph}")
            ex2_t = persist.tile([P, nm_ph], F32, name=f"ex2{ph}")
            var_t = persist.tile([P, nm_ph], F32, name=f"var{ph}")
            std_t = persist.tile([P, nm_ph], F32, name=f"std{ph}")
            inv_t = persist.tile([P, nm_ph], F32, name=f"inv{ph}")
            scale_t = persist.tile([P, nm_ph], F32, name=f"scale{ph}")
            tmp_t = persist.tile([P, nm_ph], F32, name=f"tmp{ph}")
            bias_t = persist.tile([P, nm_ph], F32, name=f"bias{ph}")

            inv_n = 1.0 / float(B_FULL)
            nc.scalar.mul(mean_t[:], stats_glob[:, 0:nm_ph], inv_n)
            nc.scalar.mul(ex2_t[:], stats_glob[:, nm_ph:], inv_n)
            nc.vector.tensor_mul(tmp_t[:], mean_t[:], mean_t[:])
            nc.vector.tensor_sub(var_t[:], ex2_t[:], tmp_t[:])
            nc.scalar.activation(std_t[:], var_t[:], AF.Sqrt, bias=eps_t[:])
            nc.vector.reciprocal(inv_t[:], std_t[:])
            g_sl = gamma_t[:, phase_m[ph][0]:phase_m[ph][-1] + 1]
            b_sl = beta_t[:, phase_m[ph][0]:phase_m[ph][-1] + 1]
            nc.vector.tensor_mul(scale_t[:], g_sl, inv_t[:])
            nc.vector.tensor_mul(tmp_t[:], mean_t[:], scale_t[:])
            nc.vector.tensor_sub(bias_t[:], b_sl, tmp_t[:])

            for m in phase_m[ph]:
                mi = m - phase_m[ph][0]
                nrm = norm_pool.tile([P, BS], F32, name="nrm")
                nc.scalar.activation(
                    nrm[:], rawp[ph][:, mi, :], AF.Identity,
                    bias=bias_t[:, mi:mi + 1], scale=scale_t[:, mi:mi + 1],
                )
                tp = tp_pool.tile([P, BS], F32, name="tp")
                nc.vector.transpose(tp[:], nrm[:])
                # tp[32B+r, 32C+c] -> out[32C+r, m*128 + 32B + c]
                for bb in range(4):
                    dsl = out_ap[:, m * P + bb * 32:m * P + (bb + 1) * 32]
                    nc.sync.dma_start(
                        dsl.rearrange("(C r) c -> r C c", r=32),
                        tp[bb * 32:(bb + 1) * 32, :].rearrange(
                            "p (C c) -> p C c", c=32),
                    )

        if do_gemm:
            emitted = set()
            gemm_all(do_drain and do_ar and do_tail)


_CACHED_NC = None


def build_nc_variant(**flags):
    nc = bacc.Bacc(
        "TRN2", target_bir_lowering=False, debug=False,
        num_devices=N_CORES,
    )
    x = nc.dram_tensor("x_shard", [BS, IN], F32, kind="ExternalInput")
    w = nc.dram_tensor("w_shard", [WPC, IN], F32, kind="ExternalInput")
    wh = nc.dram_tensor("w_head", [flags.get("nlm", NLM) * P, IN], F32,
                        kind="ExternalInput")
    gamma = nc.dram_tensor("gamma", [OUT], F32, kind="ExternalInput")
    beta = nc.dram_tensor("beta", [OUT], F32, kind="ExternalInput")
    out = nc.dram_tensor("out_shard", [BS, OUT], F32, kind="ExternalOutput")

    with tile.TileContext(nc) as tc:
        _body(nc, tc, x.ap(), w.ap(), wh.ap(), gamma.ap(), beta.ap(),
              out.ap(), **flags)

    nc.compile()
    return nc


def _build_nc():
    global _CACHED_NC
    if _CACHED_NC is None:
        _CACHED_NC = build_nc_variant()
    return _CACHED_NC


def kernel(x, weight, gamma, beta):
    x = np.ascontiguousarray(np.asarray(x, dtype=np.float32))
    weight = np.ascontiguousarray(np.asarray(weight, dtype=np.float32))
    gamma = np.ascontiguousarray(np.asarray(gamma, dtype=np.float32))
    beta = np.ascontiguousarray(np.asarray(beta, dtype=np.float32))

    nc = _build_nc()
    w_head = np.ascontiguousarray(weight[:WHR])
    in_maps = [
        {
            "x_shard": x[c * BS:(c + 1) * BS],
            "w_shard": np.ascontiguousarray(weight[c * WPC:(c + 1) * WPC]),
            "w_head": w_head,
            "gamma": gamma,
            "beta": beta,
        }
        for c in range(N_CORES)
    ]
    trace = bool(int(os.environ.get("KERNEL_TRACE", "0")))
    res = bass_utils.run_bass_kernel_spmd(
        nc, in_maps, core_ids=list(range(N_CORES)), trace=trace,
    )
    kernel.last_results = res
    return np.concatenate([r["out_shard"] for r in res.results], axis=0)
